# revision 38
# baseline (speedup 1.0000x reference)
"""Bayesian GPLVM collapsed-ELBO kernel for Trainium2 (8 NeuronCores).

Sharding: data-parallel over n (2048 rows -> 256 per core, as two
128-row chunks). Each core computes its partial psi2 (2080 upper-tri
pairs x 256 n), psi1, and A = psi1^T y; the host sums the 8 partials
and does the small m x m linear algebra to produce the scalar ELBO.
tr(y y^T), the KL statistics, and all O(n*q) elementwise input
transforms (softplus, w1/w2, ln d2, h1, the per-n exponent rows) are
input-only host work -- the device is a pure DMA -> matmul -> exp ->
reduce pipeline with nothing upstream of the first PE group.

Device flow per core:
  - One fp16 "mega" input [67, 2496]: cols 0:320 are the per-n
    exponent features (rows 0:34 psi2 side, 34:67 psi1 side, cols
    256:320 the z-side psi1 block), cols 320:2496 the psi2 z-block zl
    (34 contraction rows over 2080 upper-tri pairs; the constant-
    weighted rows are pre-reduced on host into single rows). h1 rides
    as a contraction row against the constant-1 row so the psi1 exp
    needs no bias.
  - DMA economics measured on this stack: each hardware DGE queue
    moves ~45 GB/s, serialized per queue; each dma_start costs ~600ns
    of the ISSUING engine's time; Pool's software DGE has ~10us
    latency. So: everything ships fp16; SP carries a 30KB starter
    (psi2 feature rows + zl chunk 0) plus 4-chunk zl pieces in
    consumption order; ACT carries only its 3 early issues (psi1
    rows, two y halves) before its exp stream begins.
  - psi2 runs as matmul groups of [1, 4, 4, 4, 3, 1] chunks with
    2-bank PSUM tiles and bufs=3 so PE stays ahead of ACT. Exp on ACT
    writes fp16; the n-sum is an fp16 half-add (2x DVE mode) plus a
    narrow f32-output reduce for >=3-chunk groups, a single reduce
    for smaller ones, and exp+accumulate on ACT for the two 1-chunk
    edge groups (no DVE). psi1 lives in its own 1-bank PSUM tile with
    a small exp; A = psi1^T y is two accumulated fp16 matmuls after
    group 4 whose [64, 256] copy rides ACT in the e4->e5 gap
    (Identity needs no extra table load). stats go out in two pieces
    so the final DMA is tiny.
  - The activation-table universe is filtered so table 0
    (exp_and_others) serves every Exp: the DGE-queue preload and the
    kernel's single ACT_TABLE_LOAD then reference the same table.
"""

import numpy as np

N, D, Q, M = 2048, 256, 16, 64
NCORES = 8
NLOC = N // NCORES          # 256

GROUPS = (1, 4, 4, 4, 3, 1)  # 17 pair-chunks of 128; psi1 after group 2
PSI1_GROUP = 1              # index into GROUPS
ZOFF = 320                  # zl column offset inside mega

_compiled = None


def _patch_act_tables():
    """Filter the activation-table universe handed to the table-load
    pass so the FIRST table containing Exp (table 0, exp_and_others)
    is the only one providing Exp/Ln. Table indices (act_func_set_id)
    are preserved, so the emitted load points at the real
    act_info.json entry -- and matches the table the ACT DGE queue
    preloads on its own, so one ACT_TABLE_LOAD serves the kernel."""
    import concourse.bacc as bacc_mod
    import concourse.mybir as mybir
    from concourse.hw_specs import get_activation_tables

    def patched(arch):
        real = get_activation_tables(arch)
        target = None
        for name, funcs in real.items():
            if mybir.ActivationFunctionType.Exp in funcs:
                target = name          # first = table 0, exp_and_others:
                break                  # the id the DGE preload also uses
        if target is None:
            return real
        strip = {mybir.ActivationFunctionType.Exp,
                 mybir.ActivationFunctionType.Ln}
        return {
            name: (set(funcs) if name == target else set(funcs) - strip)
            for name, funcs in real.items()
        }

    bacc_mod.get_activation_tables = patched


def _build_bass():
    import concourse.bacc as bacc
    import concourse.mybir as mybir
    from concourse.tile import TileContext

    _patch_act_tables()

    f32 = mybir.dt.float32
    f16 = mybir.dt.float16
    AF = mybir.ActivationFunctionType
    OP = mybir.AluOpType
    AX = mybir.AxisListType

    nc = bacc.Bacc("TRN2", target_bir_lowering=False, num_swdge_queues=2)

    mega_d = nc.declare_dram_parameter("mega", [67, 2496], f16, isOutput=False)
    y_d = nc.declare_dram_parameter("ybig", [128, 2 * NLOC], f16, isOutput=False)
    a_o = nc.declare_dram_parameter("out_A", [M, D], f16, isOutput=True)
    st_o = nc.declare_dram_parameter("out_stats", [128, 17], f32, isOutput=True)

    with TileContext(nc) as tc:
        with (
            tc.tile_pool(name="const", bufs=1) as cpool,
            tc.tile_pool(name="big", bufs=1) as bigpool,
            tc.tile_pool(name="scr", bufs=3) as spool,
            tc.tile_pool(name="psum", bufs=3, space="PSUM") as ppool,
            tc.tile_pool(name="psums", bufs=1, space="PSUM") as ppools,
        ):
            mega = bigpool.tile([67, 2496], f16)
            ybig = cpool.tile([128, 2 * NLOC], f16)
            # Both queues stream in parallel: SP carries the starter
            # (psi2 feature rows + zl ch0) and zl ch5-16; ACT's first
            # issue slot carries zl ch1-4 (unblocking group 2 a full
            # SP-queue turn earlier), then the psi1 rows. y halves ride
            # the tails of both queues — they are only needed by the
            # A matmul after group 4. ACT issues all precede its exps.
            nc.sync.dma_start(out=mega[0:34, 0:448], in_=mega_d[0:34, 0:448])
            nc.scalar.dma_start(out=mega[0:34, 448:960],
                                in_=mega_d[0:34, 448:960])
            nc.scalar.dma_start(out=mega[34:67, 0:320],
                                in_=mega_d[34:67, 0:320])
            for c0 in range(960, 2496, 512):
                c1 = min(c0 + 512, 2496)
                nc.sync.dma_start(out=mega[0:34, c0:c1],
                                  in_=mega_d[0:34, c0:c1])
            nc.scalar.dma_start(out=ybig[:, 0:NLOC], in_=y_d[:, 0:NLOC])
            nc.sync.dma_start(out=ybig[:, NLOC:], in_=y_d[:, NLOC:])

            stats = bigpool.tile([128, 17], f32)
            hsum = bigpool.tile([128, 512], f16)

            def psi2_group(ch0, nch):
                w = nch * NLOC
                p2 = ppool.tile([128, 4 * NLOC], f32, tag="p2")
                for j in range(nch):
                    ch = ch0 + j
                    nc.tensor.matmul(
                        p2[:, j * NLOC:(j + 1) * NLOC],
                        lhsT=mega[0:34, ZOFF + ch * 128:ZOFF + (ch + 1) * 128],
                        rhs=mega[0:34, 0:256],
                        start=True, stop=True)
                scr = spool.tile([128, 4 * NLOC], f16, tag="p2scr")
                nc.scalar.activation(scr[:, :w], p2[:, :w], AF.Exp)
                if nch >= 3:
                    # n-sum: fp16 half-add at 2x, then narrow f32 reduce
                    sv = scr[:, 0:nch * NLOC].rearrange(
                        "p (a h b) -> p a h b", h=2, b=128)
                    nc.vector.tensor_tensor(
                        out=hsum[:, 0:nch * 128].rearrange(
                            "p (a b) -> p a b", b=128),
                        in0=sv[:, :, 0, :], in1=sv[:, :, 1, :], op=OP.add)
                    nc.vector.tensor_reduce(
                        stats[:, ch0:ch0 + nch],
                        hsum[:, 0:nch * 128].rearrange(
                            "p (a b) -> p a b", b=128),
                        axis=AX.X, op=OP.add)
                else:
                    # small groups: single reduce beats add+reduce
                    nc.vector.tensor_reduce(
                        stats[:, ch0:ch0 + nch],
                        scr[:, 0:nch * NLOC].rearrange(
                            "p (a b) -> p a b", b=NLOC),
                        axis=AX.X, op=OP.add)
                return scr

            def psi2_chunk_accum(ch0):
                # single chunk as exp+accumulate on ACT: no DVE work
                p2 = ppool.tile([128, 4 * NLOC], f32, tag="p2")
                nc.tensor.matmul(
                    p2[:, 0:NLOC],
                    lhsT=mega[0:34, ZOFF + ch0 * 128:ZOFF + (ch0 + 1) * 128],
                    rhs=mega[0:34, 0:256],
                    start=True, stop=True)
                scr = spool.tile([128, 4 * NLOC], f16, tag="p2scr")
                nc.scalar.activation(scr[:, 0:NLOC], p2[:, 0:NLOC], AF.Exp,
                                     accum_out=stats[:, ch0:ch0 + 1])

            ch0 = 0
            p1scr = bigpool.tile([128, 2 * M], f16)
            for t, nch in enumerate(GROUPS):
                if nch == 1:
                    psi2_chunk_accum(ch0)
                else:
                    psi2_group(ch0, nch)
                ch0 += nch
                if t == PSI1_GROUP:
                    # psi1 in its own 1-bank PSUM tile + small exp, so
                    # every psi2 tile stays 2-bank and bufs=3 fits
                    p1p = ppools.tile([128, 2 * M], f32, tag="p1p")
                    for c in range(2):
                        nc.tensor.matmul(
                            p1p[:, M * c:M * (c + 1)],
                            lhsT=mega[0:67, 128 * c:128 * (c + 1)],
                            rhs=mega[0:67, 256:320],
                            start=True, stop=True)
                    nc.scalar.activation(p1scr[:, :], p1p[:, :], AF.Exp)
                if t == 3:
                    # stats bulk can go as soon as group 4's reduce lands
                    nc.sync.dma_start(out=st_o[:, 0:13], in_=stats[:, 0:13])
                if t == 3:
                    # A = psi1^T y, two accumulated 256-col matmuls so
                    # the output (and its copy + DMA) stays [64, 256];
                    # the copy rides ACT in the e4->e5 gap (Identity
                    # needs no extra table) and the 32KB fp16 A DMA
                    # completes well before the stats tail.
                    apsum = ppools.tile([M, D], f32, tag="aps")
                    for c in range(2):
                        nc.tensor.matmul(
                            apsum[:, :],
                            lhsT=p1scr[:, M * c:M * (c + 1)],
                            rhs=ybig[:, NLOC * c:NLOC * (c + 1)],
                            start=(c == 0), stop=(c == 1))
                    a_sb = bigpool.tile([M, D], f16)
                    nc.scalar.add(a_sb[:, :], apsum[:, :], 0.0)
                    nc.sync.dma_start(out=a_o[:, :], in_=a_sb[:, :])

            nc.sync.dma_start(out=st_o[:, 13:17], in_=stats[:, 13:17])

    nc.compile()
    return nc


def _get_compiled():
    global _compiled
    if _compiled is None:
        _compiled = _build_bass()
    return _compiled


def _np_softplus(x):
    return np.logaddexp(x, 0.0)


def kernel(y, q_mu, q_log_sigma, z, noise_raw, alpha, variance, _trace=False):
    from concourse.bass_utils import run_bass_kernel_spmd

    nc = _get_compiled()

    f8 = np.float64
    z64 = z.astype(f8)
    al = alpha.astype(f8)
    var = f8(variance[0])
    logvar = np.log(var)

    # z-side stationary block (host-built, replicated to all cores).
    # psi2 is symmetric in (i, j): ship only the 2080 upper-tri pairs.
    iu, ju = np.triu_indices(M)                             # (2080,)
    npairs = iu.shape[0]
    Su = z64[iu] + z64[ju]                                  # (2080, q)
    sqz = (z64[:, None, :] - z64[None, :, :]) ** 2          # (m, m, q)
    s1 = 0.25 * (sqz @ al)                                  # (m, m)
    zl = np.zeros((34, 17 * 128), np.float32)
    zl[0:16, :npairs] = Su.T
    zl[16:32, :npairs] = (-0.25 * Su * Su).T
    zl[32, :npairs] = 1.0
    zl[33, :npairs] = -s1[iu, ju] + 4.0 * logvar

    # per-n exponent features (host elementwise prep) + psi1 z-block
    qmu = q_mu.astype(f8)                                   # (n, q)
    qsig = _np_softplus(q_log_sigma.astype(f8))             # (n, q)
    aq = al[None, :]
    d1 = qsig * aq + 1.0
    d2 = 2.0 * qsig * aq + 1.0
    w1 = aq / d1
    w2 = aq / d2
    h1 = 2.0 * logvar - 0.5 * ((w1 * qmu * qmu).sum(-1)
                               + np.log(d1).sum(-1))        # (n,)

    feat = np.zeros((67, N), np.float32)
    feat[0:16] = (qmu * w2).T
    feat[16:32] = w2.T
    feat[32] = -((qmu * qmu * w2).sum(-1) + 0.5 * np.log(d2).sum(-1))
    feat[33] = 1.0
    feat[34:50] = (qmu * w1).T
    feat[50:66] = w1.T
    feat[66] = h1

    zt = z64.T                                              # (q, m)
    zblock = np.zeros((67, M), np.float32)
    zblock[34:50] = zt
    zblock[50:66] = -0.5 * zt * zt
    zblock[66] = 1.0

    in_maps = []
    for i in range(NCORES):
        r = i * NLOC
        mega = np.zeros((67, 2496), np.float16)
        mega[:, 0:128] = feat[:, r:r + 128]
        mega[:, 128:256] = feat[:, r + 128:r + 256]
        mega[:, 256:320] = zblock
        mega[0:34, 320:2496] = zl
        ybig = np.empty((128, 2 * NLOC), np.float16)
        ybig[:, 0:NLOC] = y[r:r + 128]
        ybig[:, NLOC:2 * NLOC] = y[r + 128:r + 256]
        in_maps.append({"mega": mega, "ybig": ybig})
    br = run_bass_kernel_spmd(nc, in_maps, list(range(NCORES)), trace=_trace)
    res = br.results

    stats = np.zeros((128, 17), f8)
    A = np.zeros((M, D), f8)
    for rr in res:
        stats += rr["out_stats"].astype(f8)
        A += rr["out_A"].astype(f8)

    flat = stats[:, 0:17].T.reshape(17 * 128)
    psi2 = np.empty((M, M), f8)
    psi2[iu, ju] = flat[:npairs]
    psi2[ju, iu] = flat[:npairs]
    tr_yy = float(np.sum(y.astype(f8) ** 2))

    # KL statistics: input-only reductions, done on host like tr_yy
    kl_sum = (-np.log(qsig).sum()
              + 0.5 * ((qsig * qsig).sum() + (qmu * qmu).sum())
              - 0.5 * N * Q)
    kl_term = kl_sum / (N * D)

    # small m x m algebra on host
    k_mm = var * np.exp(-0.5 * (sqz @ al))                  # (m, m)
    noise_var = _np_softplus(f8(noise_raw[0]))
    beta = 1.0 / noise_var
    psi0 = N * var

    cov1 = beta * psi2 + k_mm
    B = np.linalg.solve(cov1, A)
    tr_yWy = beta * tr_yy - np.sum(A * B)

    F = 0.5 * N * np.log(beta)
    F += 0.5 * np.linalg.slogdet(k_mm)[1]
    F -= 0.5 * N * np.log(np.pi)
    F -= 0.5 * np.linalg.slogdet(cov1)[1]
    F -= 0.5 * beta * psi0
    F += 0.5 * np.trace(np.linalg.solve(k_mm, psi2))
    F = (F * D - 0.5 * tr_yWy) / (N * D)

    out = F - kl_term
    result = np.asarray(out, dtype=np.float32)
    if _trace:
        return result, br
    return result


# revision 39
# speedup vs baseline: 1.0144x; 1.0144x over previous
"""Bayesian GPLVM collapsed-ELBO kernel for Trainium2 (8 NeuronCores).

Sharding: data-parallel over n (2048 rows -> 256 per core, as two
128-row chunks). Each core computes its partial psi2 (2080 upper-tri
pairs x 256 n), psi1, and A = psi1^T y; the host sums the 8 partials
and does the small m x m linear algebra to produce the scalar ELBO.
tr(y y^T), the KL statistics, and all O(n*q) elementwise input
transforms (softplus, w1/w2, ln d2, h1, the per-n exponent rows) are
input-only host work -- the device is a pure DMA -> matmul -> exp ->
reduce pipeline with nothing upstream of the first PE group.

Device flow per core:
  - One fp16 "mega" input [67, 2496]: cols 0:320 are the per-n
    exponent features (rows 0:34 psi2 side, 34:67 psi1 side, cols
    256:320 the z-side psi1 block), cols 320:2496 the psi2 z-block zl
    (34 contraction rows over 2080 upper-tri pairs; the constant-
    weighted rows are pre-reduced on host into single rows). h1 rides
    as a contraction row against the constant-1 row so the psi1 exp
    needs no bias.
  - DMA economics measured on this stack: each hardware DGE queue
    moves ~45 GB/s, serialized per queue; each dma_start costs ~600ns
    of the ISSUING engine's time; Pool's software DGE has ~10us
    latency. So: everything ships fp16; SP carries a 30KB starter
    (psi2 feature rows + zl chunk 0) plus 4-chunk zl pieces in
    consumption order; ACT carries only its 3 early issues (psi1
    rows, two y halves) before its exp stream begins.
  - psi2 runs as matmul groups of [1, 4, 4, 4, 3, 1] chunks with
    2-bank PSUM tiles and bufs=3 so PE stays ahead of ACT. Exp on ACT
    writes fp16; the n-sum is an fp16 half-add (2x DVE mode) plus a
    narrow f32-output reduce for >=3-chunk groups, a single reduce
    for smaller ones, and exp+accumulate on ACT for the two 1-chunk
    edge groups (no DVE). psi1 lives in its own 1-bank PSUM tile with
    a small exp; A = psi1^T y is two accumulated fp16 matmuls after
    group 4 whose [64, 256] copy rides ACT in the e4->e5 gap
    (Identity needs no extra table load). stats go out in two pieces
    so the final DMA is tiny.
  - The activation-table universe is filtered so table 0
    (exp_and_others) serves every Exp: the DGE-queue preload and the
    kernel's single ACT_TABLE_LOAD then reference the same table.
"""

import numpy as np

N, D, Q, M = 2048, 256, 16, 64
NCORES = 8
NLOC = N // NCORES          # 256

GROUPS = (1, 4, 4, 4, 3, 1)  # 17 pair-chunks of 128; psi1 after group 2
PSI1_GROUP = 1              # index into GROUPS
ZOFF = 320                  # zl column offset inside mega

_compiled = None


def _patch_act_tables():
    """Filter the activation-table universe handed to the table-load
    pass so the FIRST table containing Exp (table 0, exp_and_others)
    is the only one providing Exp/Ln. Table indices (act_func_set_id)
    are preserved, so the emitted load points at the real
    act_info.json entry -- and matches the table the ACT DGE queue
    preloads on its own, so one ACT_TABLE_LOAD serves the kernel."""
    import concourse.bacc as bacc_mod
    import concourse.mybir as mybir
    from concourse.hw_specs import get_activation_tables

    def patched(arch):
        real = get_activation_tables(arch)
        target = None
        for name, funcs in real.items():
            if mybir.ActivationFunctionType.Exp in funcs:
                target = name          # first = table 0, exp_and_others:
                break                  # the id the DGE preload also uses
        if target is None:
            return real
        strip = {mybir.ActivationFunctionType.Exp,
                 mybir.ActivationFunctionType.Ln}
        return {
            name: (set(funcs) if name == target else set(funcs) - strip)
            for name, funcs in real.items()
        }

    bacc_mod.get_activation_tables = patched


def _build_bass():
    import concourse.bacc as bacc
    import concourse.mybir as mybir
    from concourse.tile import TileContext

    _patch_act_tables()

    f32 = mybir.dt.float32
    f16 = mybir.dt.float16
    AF = mybir.ActivationFunctionType
    OP = mybir.AluOpType
    AX = mybir.AxisListType

    nc = bacc.Bacc("TRN2", target_bir_lowering=False, num_swdge_queues=2)

    mega_d = nc.declare_dram_parameter("mega", [67, 2496], f16, isOutput=False)
    y_d = nc.declare_dram_parameter("ybig", [128, 2 * NLOC], f16, isOutput=False)
    a_o = nc.declare_dram_parameter("out_A", [M, D], f16, isOutput=True)
    st_o = nc.declare_dram_parameter("out_stats", [128, 17], f32, isOutput=True)

    with TileContext(nc) as tc:
        with (
            tc.tile_pool(name="const", bufs=1) as cpool,
            tc.tile_pool(name="big", bufs=1) as bigpool,
            tc.tile_pool(name="scr", bufs=3) as spool,
            tc.tile_pool(name="psum", bufs=3, space="PSUM") as ppool,
            tc.tile_pool(name="psums", bufs=1, space="PSUM") as ppools,
        ):
            mega = bigpool.tile([67, 2496], f16)
            ybig = cpool.tile([128, 2 * NLOC], f16)
            # SP queue: starter pack (psi2 feature rows + zl ch0),
            # then zl in 4-chunk pieces in consumption order. ACT
            # affords only its 3 early issues (each DMA issue costs
            # ~600ns of the issuing ENGINE): psi1 rows, then y halves.
            nc.sync.dma_start(out=mega[0:34, 0:448], in_=mega_d[0:34, 0:448])
            nc.scalar.dma_start(out=mega[34:67, 0:320],
                                in_=mega_d[34:67, 0:320])
            for c0 in range(448, 2496, 512):
                c1 = min(c0 + 512, 2496)
                nc.sync.dma_start(out=mega[0:34, c0:c1],
                                  in_=mega_d[0:34, c0:c1])
            nc.scalar.dma_start(out=ybig[:, 0:NLOC], in_=y_d[:, 0:NLOC])
            nc.scalar.dma_start(out=ybig[:, NLOC:], in_=y_d[:, NLOC:])

            stats = bigpool.tile([128, 17], f32)
            hsum = bigpool.tile([128, 512], f16)

            def psi2_group(ch0, nch):
                w = nch * NLOC
                p2 = ppool.tile([128, 4 * NLOC], f32, tag="p2")
                for j in range(nch):
                    ch = ch0 + j
                    nc.tensor.matmul(
                        p2[:, j * NLOC:(j + 1) * NLOC],
                        lhsT=mega[0:34, ZOFF + ch * 128:ZOFF + (ch + 1) * 128],
                        rhs=mega[0:34, 0:256],
                        start=True, stop=True)
                scr = spool.tile([128, 4 * NLOC], f16, tag="p2scr")
                nc.scalar.activation(scr[:, :w], p2[:, :w], AF.Exp)
                if nch >= 3:
                    # n-sum: fp16 half-add at 2x, then narrow f32 reduce
                    sv = scr[:, 0:nch * NLOC].rearrange(
                        "p (a h b) -> p a h b", h=2, b=128)
                    nc.vector.tensor_tensor(
                        out=hsum[:, 0:nch * 128].rearrange(
                            "p (a b) -> p a b", b=128),
                        in0=sv[:, :, 0, :], in1=sv[:, :, 1, :], op=OP.add)
                    nc.vector.tensor_reduce(
                        stats[:, ch0:ch0 + nch],
                        hsum[:, 0:nch * 128].rearrange(
                            "p (a b) -> p a b", b=128),
                        axis=AX.X, op=OP.add)
                else:
                    # small groups: single reduce beats add+reduce
                    nc.vector.tensor_reduce(
                        stats[:, ch0:ch0 + nch],
                        scr[:, 0:nch * NLOC].rearrange(
                            "p (a b) -> p a b", b=NLOC),
                        axis=AX.X, op=OP.add)
                return scr

            def psi2_chunk_accum(ch0):
                # single chunk as exp+accumulate on ACT: no DVE work
                p2 = ppool.tile([128, 4 * NLOC], f32, tag="p2")
                nc.tensor.matmul(
                    p2[:, 0:NLOC],
                    lhsT=mega[0:34, ZOFF + ch0 * 128:ZOFF + (ch0 + 1) * 128],
                    rhs=mega[0:34, 0:256],
                    start=True, stop=True)
                scr = spool.tile([128, 4 * NLOC], f16, tag="p2scr")
                nc.scalar.activation(scr[:, 0:NLOC], p2[:, 0:NLOC], AF.Exp,
                                     accum_out=stats[:, ch0:ch0 + 1])

            ch0 = 0
            p1scr = bigpool.tile([128, 2 * M], f16)
            for t, nch in enumerate(GROUPS):
                if nch == 1:
                    psi2_chunk_accum(ch0)
                else:
                    psi2_group(ch0, nch)
                ch0 += nch
                if t == PSI1_GROUP:
                    # psi1 in its own 1-bank PSUM tile + small exp, so
                    # every psi2 tile stays 2-bank and bufs=3 fits
                    p1p = ppools.tile([128, 2 * M], f32, tag="p1p")
                    for c in range(2):
                        nc.tensor.matmul(
                            p1p[:, M * c:M * (c + 1)],
                            lhsT=mega[0:67, 128 * c:128 * (c + 1)],
                            rhs=mega[0:67, 256:320],
                            start=True, stop=True)
                    nc.scalar.activation(p1scr[:, :], p1p[:, :], AF.Exp)
                if t == 3:
                    # stats bulk can go as soon as group 4's reduce lands
                    nc.sync.dma_start(out=st_o[:, 0:13], in_=stats[:, 0:13])
                if t == 3:
                    # A = psi1^T y, two accumulated 256-col matmuls so
                    # the output (and its copy + DMA) stays [64, 256];
                    # the copy rides ACT in the e4->e5 gap (Identity
                    # needs no extra table) and the 32KB fp16 A DMA
                    # completes well before the stats tail.
                    apsum = ppools.tile([M, D], f32, tag="aps")
                    for c in range(2):
                        nc.tensor.matmul(
                            apsum[:, :],
                            lhsT=p1scr[:, M * c:M * (c + 1)],
                            rhs=ybig[:, NLOC * c:NLOC * (c + 1)],
                            start=(c == 0), stop=(c == 1))
                    a_sb = bigpool.tile([M, D], f16)
                    nc.scalar.add(a_sb[:, :], apsum[:, :], 0.0)
                    nc.sync.dma_start(out=a_o[:, :], in_=a_sb[:, :])

            nc.sync.dma_start(out=st_o[:, 13:17], in_=stats[:, 13:17])

    nc.compile()
    return nc


def _get_compiled():
    global _compiled
    if _compiled is None:
        _compiled = _build_bass()
    return _compiled


def _np_softplus(x):
    return np.logaddexp(x, 0.0)


def kernel(y, q_mu, q_log_sigma, z, noise_raw, alpha, variance, _trace=False):
    from concourse.bass_utils import run_bass_kernel_spmd

    nc = _get_compiled()

    f8 = np.float64
    z64 = z.astype(f8)
    al = alpha.astype(f8)
    var = f8(variance[0])
    logvar = np.log(var)

    # z-side stationary block (host-built, replicated to all cores).
    # psi2 is symmetric in (i, j): ship only the 2080 upper-tri pairs.
    iu, ju = np.triu_indices(M)                             # (2080,)
    npairs = iu.shape[0]
    Su = z64[iu] + z64[ju]                                  # (2080, q)
    sqz = (z64[:, None, :] - z64[None, :, :]) ** 2          # (m, m, q)
    s1 = 0.25 * (sqz @ al)                                  # (m, m)
    zl = np.zeros((34, 17 * 128), np.float32)
    zl[0:16, :npairs] = Su.T
    zl[16:32, :npairs] = (-0.25 * Su * Su).T
    zl[32, :npairs] = 1.0
    zl[33, :npairs] = -s1[iu, ju] + 4.0 * logvar

    # per-n exponent features (host elementwise prep) + psi1 z-block
    qmu = q_mu.astype(f8)                                   # (n, q)
    qsig = _np_softplus(q_log_sigma.astype(f8))             # (n, q)
    aq = al[None, :]
    d1 = qsig * aq + 1.0
    d2 = 2.0 * qsig * aq + 1.0
    w1 = aq / d1
    w2 = aq / d2
    h1 = 2.0 * logvar - 0.5 * ((w1 * qmu * qmu).sum(-1)
                               + np.log(d1).sum(-1))        # (n,)

    feat = np.zeros((67, N), np.float32)
    feat[0:16] = (qmu * w2).T
    feat[16:32] = w2.T
    feat[32] = -((qmu * qmu * w2).sum(-1) + 0.5 * np.log(d2).sum(-1))
    feat[33] = 1.0
    feat[34:50] = (qmu * w1).T
    feat[50:66] = w1.T
    feat[66] = h1

    zt = z64.T                                              # (q, m)
    zblock = np.zeros((67, M), np.float32)
    zblock[34:50] = zt
    zblock[50:66] = -0.5 * zt * zt
    zblock[66] = 1.0

    in_maps = []
    for i in range(NCORES):
        r = i * NLOC
        mega = np.zeros((67, 2496), np.float16)
        mega[:, 0:128] = feat[:, r:r + 128]
        mega[:, 128:256] = feat[:, r + 128:r + 256]
        mega[:, 256:320] = zblock
        mega[0:34, 320:2496] = zl
        ybig = np.empty((128, 2 * NLOC), np.float16)
        ybig[:, 0:NLOC] = y[r:r + 128]
        ybig[:, NLOC:2 * NLOC] = y[r + 128:r + 256]
        in_maps.append({"mega": mega, "ybig": ybig})
    br = run_bass_kernel_spmd(nc, in_maps, list(range(NCORES)), trace=_trace)
    res = br.results

    stats = np.zeros((128, 17), f8)
    A = np.zeros((M, D), f8)
    for rr in res:
        stats += rr["out_stats"].astype(f8)
        A += rr["out_A"].astype(f8)

    flat = stats[:, 0:17].T.reshape(17 * 128)
    psi2 = np.empty((M, M), f8)
    psi2[iu, ju] = flat[:npairs]
    psi2[ju, iu] = flat[:npairs]
    tr_yy = float(np.sum(y.astype(f8) ** 2))

    # KL statistics: input-only reductions, done on host like tr_yy
    kl_sum = (-np.log(qsig).sum()
              + 0.5 * ((qsig * qsig).sum() + (qmu * qmu).sum())
              - 0.5 * N * Q)
    kl_term = kl_sum / (N * D)

    # small m x m algebra on host
    k_mm = var * np.exp(-0.5 * (sqz @ al))                  # (m, m)
    noise_var = _np_softplus(f8(noise_raw[0]))
    beta = 1.0 / noise_var
    psi0 = N * var

    cov1 = beta * psi2 + k_mm
    B = np.linalg.solve(cov1, A)
    tr_yWy = beta * tr_yy - np.sum(A * B)

    F = 0.5 * N * np.log(beta)
    F += 0.5 * np.linalg.slogdet(k_mm)[1]
    F -= 0.5 * N * np.log(np.pi)
    F -= 0.5 * np.linalg.slogdet(cov1)[1]
    F -= 0.5 * beta * psi0
    F += 0.5 * np.trace(np.linalg.solve(k_mm, psi2))
    F = (F * D - 0.5 * tr_yWy) / (N * D)

    out = F - kl_term
    result = np.asarray(out, dtype=np.float32)
    if _trace:
        return result, br
    return result


# revision 40
# speedup vs baseline: 1.0236x; 1.0091x over previous
"""Bayesian GPLVM collapsed-ELBO kernel for Trainium2 (8 NeuronCores).

Sharding: data-parallel over n (2048 rows -> 256 per core, as two
128-row chunks). Each core computes its partial psi2 (2080 upper-tri
pairs x 256 n), psi1, and A = psi1^T y; the host sums the 8 partials
and does the small m x m linear algebra to produce the scalar ELBO.
tr(y y^T), the KL statistics, and all O(n*q) elementwise input
transforms (softplus, w1/w2, ln d2, h1, the per-n exponent rows) are
input-only host work -- the device is a pure DMA -> matmul -> exp ->
reduce pipeline with nothing upstream of the first PE group.

Device flow per core:
  - One fp16 "mega" input [67, 2496]: cols 0:320 are the per-n
    exponent features (rows 0:34 psi2 side, 34:67 psi1 side, cols
    256:320 the z-side psi1 block), cols 320:2496 the psi2 z-block zl
    (34 contraction rows over 2080 upper-tri pairs; the constant-
    weighted rows are pre-reduced on host into single rows). h1 rides
    as a contraction row against the constant-1 row so the psi1 exp
    needs no bias.
  - DMA economics measured on this stack: each hardware DGE queue
    moves ~45 GB/s, serialized per queue; each dma_start costs ~600ns
    of the ISSUING engine's time; Pool's software DGE has ~10us
    latency. So: everything ships fp16; SP carries a 30KB starter
    (psi2 feature rows + zl chunk 0) plus 4-chunk zl pieces in
    consumption order; ACT carries only its 3 early issues (psi1
    rows, two y halves) before its exp stream begins.
  - psi2 runs as matmul groups of [1, 4, 4, 4, 3, 1] chunks with
    2-bank PSUM tiles and bufs=3 so PE stays ahead of ACT. Exp on ACT
    writes fp16; the n-sum is an fp16 half-add (2x DVE mode) plus a
    narrow f32-output reduce for >=3-chunk groups, a single reduce
    for smaller ones, and exp+accumulate on ACT for the two 1-chunk
    edge groups (no DVE). psi1 lives in its own 1-bank PSUM tile with
    a small exp; A = psi1^T y is two accumulated fp16 matmuls after
    group 4 whose [64, 256] copy rides ACT in the e4->e5 gap
    (Identity needs no extra table load). stats go out in two pieces
    so the final DMA is tiny.
  - The activation-table universe is filtered so table 0
    (exp_and_others) serves every Exp: the DGE-queue preload and the
    kernel's single ACT_TABLE_LOAD then reference the same table.
"""

import numpy as np

N, D, Q, M = 2048, 256, 16, 64
NCORES = 8
NLOC = N // NCORES          # 256

GROUPS = (2, 4, 4, 4, 2, 1)  # 17 pair-chunks of 128; psi1 after group 2
PSI1_GROUP = 1              # index into GROUPS
ZOFF = 320                  # zl column offset inside mega

_compiled = None


def _patch_act_tables():
    """Filter the activation-table universe handed to the table-load
    pass so the FIRST table containing Exp (table 0, exp_and_others)
    is the only one providing Exp/Ln. Table indices (act_func_set_id)
    are preserved, so the emitted load points at the real
    act_info.json entry -- and matches the table the ACT DGE queue
    preloads on its own, so one ACT_TABLE_LOAD serves the kernel."""
    import concourse.bacc as bacc_mod
    import concourse.mybir as mybir
    from concourse.hw_specs import get_activation_tables

    def patched(arch):
        real = get_activation_tables(arch)
        target = None
        for name, funcs in real.items():
            if mybir.ActivationFunctionType.Exp in funcs:
                target = name          # first = table 0, exp_and_others:
                break                  # the id the DGE preload also uses
        if target is None:
            return real
        strip = {mybir.ActivationFunctionType.Exp,
                 mybir.ActivationFunctionType.Ln}
        return {
            name: (set(funcs) if name == target else set(funcs) - strip)
            for name, funcs in real.items()
        }

    bacc_mod.get_activation_tables = patched


def _build_bass():
    import concourse.bacc as bacc
    import concourse.mybir as mybir
    from concourse.tile import TileContext

    _patch_act_tables()

    f32 = mybir.dt.float32
    f16 = mybir.dt.float16
    AF = mybir.ActivationFunctionType
    OP = mybir.AluOpType
    AX = mybir.AxisListType

    nc = bacc.Bacc("TRN2", target_bir_lowering=False, num_swdge_queues=2)

    mega_d = nc.declare_dram_parameter("mega", [67, 2496], f16, isOutput=False)
    y_d = nc.declare_dram_parameter("ybig", [128, 2 * NLOC], f16, isOutput=False)
    a_o = nc.declare_dram_parameter("out_A", [M, D], f16, isOutput=True)
    st_o = nc.declare_dram_parameter("out_stats", [128, 17], f32, isOutput=True)

    with TileContext(nc) as tc:
        with (
            tc.tile_pool(name="const", bufs=1) as cpool,
            tc.tile_pool(name="big", bufs=1) as bigpool,
            tc.tile_pool(name="scr", bufs=3) as spool,
            tc.tile_pool(name="psum", bufs=3, space="PSUM") as ppool,
            tc.tile_pool(name="psums", bufs=1, space="PSUM") as ppools,
        ):
            mega = bigpool.tile([67, 2496], f16)
            ybig = cpool.tile([128, 2 * NLOC], f16)
            # SP queue: starter pack (psi2 feature rows + zl ch0),
            # then zl in 4-chunk pieces in consumption order. ACT
            # affords only its 3 early issues (each DMA issue costs
            # ~600ns of the issuing ENGINE): psi1 rows, then y halves.
            nc.sync.dma_start(out=mega[0:34, 0:576], in_=mega_d[0:34, 0:576])
            nc.scalar.dma_start(out=mega[34:67, 0:320],
                                in_=mega_d[34:67, 0:320])
            for c0 in range(576, 2496, 512):
                c1 = min(c0 + 512, 2496)
                nc.sync.dma_start(out=mega[0:34, c0:c1],
                                  in_=mega_d[0:34, c0:c1])
            nc.scalar.dma_start(out=ybig[:, 0:NLOC], in_=y_d[:, 0:NLOC])
            nc.scalar.dma_start(out=ybig[:, NLOC:], in_=y_d[:, NLOC:])

            stats = bigpool.tile([128, 17], f32)
            hsum = bigpool.tile([128, 512], f16)

            def psi2_group(ch0, nch):
                w = nch * NLOC
                p2 = ppool.tile([128, 4 * NLOC], f32, tag="p2")
                for j in range(nch):
                    ch = ch0 + j
                    nc.tensor.matmul(
                        p2[:, j * NLOC:(j + 1) * NLOC],
                        lhsT=mega[0:34, ZOFF + ch * 128:ZOFF + (ch + 1) * 128],
                        rhs=mega[0:34, 0:256],
                        start=True, stop=True)
                scr = spool.tile([128, 4 * NLOC], f16, tag="p2scr")
                nc.scalar.activation(scr[:, :w], p2[:, :w], AF.Exp)
                if nch >= 3:
                    # n-sum: fp16 half-add at 2x, then narrow f32 reduce
                    sv = scr[:, 0:nch * NLOC].rearrange(
                        "p (a h b) -> p a h b", h=2, b=128)
                    nc.vector.tensor_tensor(
                        out=hsum[:, 0:nch * 128].rearrange(
                            "p (a b) -> p a b", b=128),
                        in0=sv[:, :, 0, :], in1=sv[:, :, 1, :], op=OP.add)
                    nc.vector.tensor_reduce(
                        stats[:, ch0:ch0 + nch],
                        hsum[:, 0:nch * 128].rearrange(
                            "p (a b) -> p a b", b=128),
                        axis=AX.X, op=OP.add)
                else:
                    # small groups: single reduce beats add+reduce
                    nc.vector.tensor_reduce(
                        stats[:, ch0:ch0 + nch],
                        scr[:, 0:nch * NLOC].rearrange(
                            "p (a b) -> p a b", b=NLOC),
                        axis=AX.X, op=OP.add)
                return scr

            def psi2_chunk_accum(ch0):
                # single chunk as exp+accumulate on ACT: no DVE work
                p2 = ppool.tile([128, 4 * NLOC], f32, tag="p2")
                nc.tensor.matmul(
                    p2[:, 0:NLOC],
                    lhsT=mega[0:34, ZOFF + ch0 * 128:ZOFF + (ch0 + 1) * 128],
                    rhs=mega[0:34, 0:256],
                    start=True, stop=True)
                scr = spool.tile([128, 4 * NLOC], f16, tag="p2scr")
                nc.scalar.activation(scr[:, 0:NLOC], p2[:, 0:NLOC], AF.Exp,
                                     accum_out=stats[:, ch0:ch0 + 1])

            ch0 = 0
            p1scr = bigpool.tile([128, 2 * M], f16)
            for t, nch in enumerate(GROUPS):
                if nch == 1:
                    psi2_chunk_accum(ch0)
                else:
                    psi2_group(ch0, nch)
                ch0 += nch
                if t == PSI1_GROUP:
                    # psi1 in its own 1-bank PSUM tile + small exp, so
                    # every psi2 tile stays 2-bank and bufs=3 fits
                    p1p = ppools.tile([128, 2 * M], f32, tag="p1p")
                    for c in range(2):
                        nc.tensor.matmul(
                            p1p[:, M * c:M * (c + 1)],
                            lhsT=mega[0:67, 128 * c:128 * (c + 1)],
                            rhs=mega[0:67, 256:320],
                            start=True, stop=True)
                    nc.scalar.activation(p1scr[:, :], p1p[:, :], AF.Exp)
                if t == 3:
                    # stats bulk can go as soon as group 4's reduce lands
                    nc.sync.dma_start(out=st_o[:, 0:14], in_=stats[:, 0:14])
                if t == 3:
                    # A = psi1^T y, two accumulated 256-col matmuls so
                    # the output (and its copy + DMA) stays [64, 256];
                    # the copy rides ACT in the e4->e5 gap (Identity
                    # needs no extra table) and the 32KB fp16 A DMA
                    # completes well before the stats tail.
                    apsum = ppools.tile([M, D], f32, tag="aps")
                    for c in range(2):
                        nc.tensor.matmul(
                            apsum[:, :],
                            lhsT=p1scr[:, M * c:M * (c + 1)],
                            rhs=ybig[:, NLOC * c:NLOC * (c + 1)],
                            start=(c == 0), stop=(c == 1))
                    a_sb = bigpool.tile([M, D], f16)
                    nc.scalar.add(a_sb[:, :], apsum[:, :], 0.0)
                    nc.sync.dma_start(out=a_o[:, :], in_=a_sb[:, :])

            nc.sync.dma_start(out=st_o[:, 14:17], in_=stats[:, 14:17])

    nc.compile()
    return nc


def _get_compiled():
    global _compiled
    if _compiled is None:
        _compiled = _build_bass()
    return _compiled


def _np_softplus(x):
    return np.logaddexp(x, 0.0)


def kernel(y, q_mu, q_log_sigma, z, noise_raw, alpha, variance, _trace=False):
    from concourse.bass_utils import run_bass_kernel_spmd

    nc = _get_compiled()

    f8 = np.float64
    z64 = z.astype(f8)
    al = alpha.astype(f8)
    var = f8(variance[0])
    logvar = np.log(var)

    # z-side stationary block (host-built, replicated to all cores).
    # psi2 is symmetric in (i, j): ship only the 2080 upper-tri pairs.
    iu, ju = np.triu_indices(M)                             # (2080,)
    npairs = iu.shape[0]
    Su = z64[iu] + z64[ju]                                  # (2080, q)
    sqz = (z64[:, None, :] - z64[None, :, :]) ** 2          # (m, m, q)
    s1 = 0.25 * (sqz @ al)                                  # (m, m)
    zl = np.zeros((34, 17 * 128), np.float32)
    zl[0:16, :npairs] = Su.T
    zl[16:32, :npairs] = (-0.25 * Su * Su).T
    zl[32, :npairs] = 1.0
    zl[33, :npairs] = -s1[iu, ju] + 4.0 * logvar

    # per-n exponent features (host elementwise prep) + psi1 z-block
    qmu = q_mu.astype(f8)                                   # (n, q)
    qsig = _np_softplus(q_log_sigma.astype(f8))             # (n, q)
    aq = al[None, :]
    d1 = qsig * aq + 1.0
    d2 = 2.0 * qsig * aq + 1.0
    w1 = aq / d1
    w2 = aq / d2
    h1 = 2.0 * logvar - 0.5 * ((w1 * qmu * qmu).sum(-1)
                               + np.log(d1).sum(-1))        # (n,)

    feat = np.zeros((67, N), np.float32)
    feat[0:16] = (qmu * w2).T
    feat[16:32] = w2.T
    feat[32] = -((qmu * qmu * w2).sum(-1) + 0.5 * np.log(d2).sum(-1))
    feat[33] = 1.0
    feat[34:50] = (qmu * w1).T
    feat[50:66] = w1.T
    feat[66] = h1

    zt = z64.T                                              # (q, m)
    zblock = np.zeros((67, M), np.float32)
    zblock[34:50] = zt
    zblock[50:66] = -0.5 * zt * zt
    zblock[66] = 1.0

    in_maps = []
    for i in range(NCORES):
        r = i * NLOC
        mega = np.zeros((67, 2496), np.float16)
        mega[:, 0:128] = feat[:, r:r + 128]
        mega[:, 128:256] = feat[:, r + 128:r + 256]
        mega[:, 256:320] = zblock
        mega[0:34, 320:2496] = zl
        ybig = np.empty((128, 2 * NLOC), np.float16)
        ybig[:, 0:NLOC] = y[r:r + 128]
        ybig[:, NLOC:2 * NLOC] = y[r + 128:r + 256]
        in_maps.append({"mega": mega, "ybig": ybig})
    br = run_bass_kernel_spmd(nc, in_maps, list(range(NCORES)), trace=_trace)
    res = br.results

    stats = np.zeros((128, 17), f8)
    A = np.zeros((M, D), f8)
    for rr in res:
        stats += rr["out_stats"].astype(f8)
        A += rr["out_A"].astype(f8)

    flat = stats[:, 0:17].T.reshape(17 * 128)
    psi2 = np.empty((M, M), f8)
    psi2[iu, ju] = flat[:npairs]
    psi2[ju, iu] = flat[:npairs]
    tr_yy = float(np.sum(y.astype(f8) ** 2))

    # KL statistics: input-only reductions, done on host like tr_yy
    kl_sum = (-np.log(qsig).sum()
              + 0.5 * ((qsig * qsig).sum() + (qmu * qmu).sum())
              - 0.5 * N * Q)
    kl_term = kl_sum / (N * D)

    # small m x m algebra on host
    k_mm = var * np.exp(-0.5 * (sqz @ al))                  # (m, m)
    noise_var = _np_softplus(f8(noise_raw[0]))
    beta = 1.0 / noise_var
    psi0 = N * var

    cov1 = beta * psi2 + k_mm
    B = np.linalg.solve(cov1, A)
    tr_yWy = beta * tr_yy - np.sum(A * B)

    F = 0.5 * N * np.log(beta)
    F += 0.5 * np.linalg.slogdet(k_mm)[1]
    F -= 0.5 * N * np.log(np.pi)
    F -= 0.5 * np.linalg.slogdet(cov1)[1]
    F -= 0.5 * beta * psi0
    F += 0.5 * np.trace(np.linalg.solve(k_mm, psi2))
    F = (F * D - 0.5 * tr_yWy) / (N * D)

    out = F - kl_term
    result = np.asarray(out, dtype=np.float32)
    if _trace:
        return result, br
    return result


# revision 41
# speedup vs baseline: 1.0350x; 1.0111x over previous
"""Bayesian GPLVM collapsed-ELBO kernel for Trainium2 (8 NeuronCores).

Sharding: data-parallel over n (2048 rows -> 256 per core, as two
128-row chunks). Each core computes its partial psi2 (2080 upper-tri
pairs x 256 n), psi1, and A = psi1^T y; the host sums the 8 partials
and does the small m x m linear algebra to produce the scalar ELBO.
tr(y y^T), the KL statistics, and all O(n*q) elementwise input
transforms (softplus, w1/w2, ln d2, h1, the per-n exponent rows) are
input-only host work -- the device is a pure DMA -> matmul -> exp ->
reduce pipeline with nothing upstream of the first PE group.

Device flow per core:
  - One fp16 "mega" input [67, 2496]: cols 0:320 are the per-n
    exponent features (rows 0:34 psi2 side, 34:67 psi1 side, cols
    256:320 the z-side psi1 block), cols 320:2496 the psi2 z-block zl
    (34 contraction rows over 2080 upper-tri pairs; the constant-
    weighted rows are pre-reduced on host into single rows). h1 rides
    as a contraction row against the constant-1 row so the psi1 exp
    needs no bias.
  - DMA economics measured on this stack: each hardware DGE queue
    moves ~45 GB/s, serialized per queue; each dma_start costs ~600ns
    of the ISSUING engine's time; Pool's software DGE has ~10us
    latency; on top, each transfer pays ~1us of fixed queue time, so
    the 322KB of fp16 inputs across 8 transfers make the pipeline
    input-bandwidth-bound end to end. SP carries a 39KB starter (psi2
    feature rows + zl chunks 0-1) plus 4-chunk zl pieces in
    consumption order; ACT carries only its 3 early issues (psi1
    rows, two y halves) before its exp stream begins.
  - psi2 runs as matmul groups of [2, 4, 4, 4, 2, 1] chunks with
    2-bank PSUM tiles and bufs=3 so PE stays ahead of ACT. Exp on ACT
    writes fp16; the n-sum is an fp16 half-add (2x DVE mode) plus a
    narrow f32-output reduce for >=3-chunk groups, a single reduce
    for 2-chunk ones, and exp+accumulate on ACT for the final chunk
    (no DVE at the tail). psi1 lives in its own 1-bank PSUM tile with
    a small exp; A = psi1^T y is two accumulated fp16 matmuls after
    group 4 whose [64, 256] copy rides ACT in the e4->e5 gap
    (Identity needs no extra table load). stats go out in two pieces
    so the final DMA is tiny.
  - The activation-table universe is filtered so table 0
    (exp_and_others) serves every Exp: the DGE-queue preload and the
    kernel's single ACT_TABLE_LOAD then reference the same table.
"""

import numpy as np

N, D, Q, M = 2048, 256, 16, 64
NCORES = 8
NLOC = N // NCORES          # 256

GROUPS = (2, 4, 4, 4, 2, 1)  # 17 pair-chunks of 128; psi1 after group 2
PSI1_GROUP = 1              # index into GROUPS
ZOFF = 320                  # zl column offset inside mega

_compiled = None


def _patch_act_tables():
    """Filter the activation-table universe handed to the table-load
    pass so the FIRST table containing Exp (table 0, exp_and_others)
    is the only one providing Exp/Ln. Table indices (act_func_set_id)
    are preserved, so the emitted load points at the real
    act_info.json entry -- and matches the table the ACT DGE queue
    preloads on its own, so one ACT_TABLE_LOAD serves the kernel."""
    import concourse.bacc as bacc_mod
    import concourse.mybir as mybir
    from concourse.hw_specs import get_activation_tables

    def patched(arch):
        real = get_activation_tables(arch)
        target = None
        for name, funcs in real.items():
            if mybir.ActivationFunctionType.Exp in funcs:
                target = name          # first = table 0, exp_and_others:
                break                  # the id the DGE preload also uses
        if target is None:
            return real
        strip = {mybir.ActivationFunctionType.Exp,
                 mybir.ActivationFunctionType.Ln}
        return {
            name: (set(funcs) if name == target else set(funcs) - strip)
            for name, funcs in real.items()
        }

    bacc_mod.get_activation_tables = patched


def _build_bass():
    import concourse.bacc as bacc
    import concourse.mybir as mybir
    from concourse.tile import TileContext

    _patch_act_tables()

    f32 = mybir.dt.float32
    f16 = mybir.dt.float16
    AF = mybir.ActivationFunctionType
    OP = mybir.AluOpType
    AX = mybir.AxisListType

    nc = bacc.Bacc("TRN2", target_bir_lowering=False, num_swdge_queues=2)

    mega_d = nc.declare_dram_parameter("mega", [67, 2496], f16, isOutput=False)
    y_d = nc.declare_dram_parameter("ybig", [128, 2 * NLOC], f16, isOutput=False)
    a_o = nc.declare_dram_parameter("out_A", [M, D], f16, isOutput=True)
    st_o = nc.declare_dram_parameter("out_stats", [128, 17], f32, isOutput=True)

    with TileContext(nc) as tc:
        with (
            tc.tile_pool(name="const", bufs=1) as cpool,
            tc.tile_pool(name="big", bufs=1) as bigpool,
            tc.tile_pool(name="scr", bufs=3) as spool,
            tc.tile_pool(name="psum", bufs=3, space="PSUM") as ppool,
            tc.tile_pool(name="psums", bufs=1, space="PSUM") as ppools,
        ):
            mega = bigpool.tile([67, 2496], f16)
            ybig = cpool.tile([128, 2 * NLOC], f16)
            # SP queue: starter pack (psi2 feature rows + zl ch0),
            # then zl in 4-chunk pieces in consumption order. ACT
            # affords only its 3 early issues (each DMA issue costs
            # ~600ns of the issuing ENGINE): psi1 rows, then y halves.
            nc.sync.dma_start(out=mega[0:34, 0:576], in_=mega_d[0:34, 0:576])
            nc.scalar.dma_start(out=mega[34:67, 0:320],
                                in_=mega_d[34:67, 0:320])
            for c0 in range(576, 2496, 512):
                c1 = min(c0 + 512, 2496)
                nc.sync.dma_start(out=mega[0:34, c0:c1],
                                  in_=mega_d[0:34, c0:c1])
            nc.scalar.dma_start(out=ybig[:, 0:NLOC], in_=y_d[:, 0:NLOC])
            nc.scalar.dma_start(out=ybig[:, NLOC:], in_=y_d[:, NLOC:])

            stats = bigpool.tile([128, 17], f32)
            hsum = bigpool.tile([128, 512], f16)

            def psi2_group(ch0, nch):
                w = nch * NLOC
                p2 = ppool.tile([128, 4 * NLOC], f32, tag="p2")
                for j in range(nch):
                    ch = ch0 + j
                    nc.tensor.matmul(
                        p2[:, j * NLOC:(j + 1) * NLOC],
                        lhsT=mega[0:34, ZOFF + ch * 128:ZOFF + (ch + 1) * 128],
                        rhs=mega[0:34, 0:256],
                        start=True, stop=True)
                scr = spool.tile([128, 4 * NLOC], f16, tag="p2scr")
                nc.scalar.activation(scr[:, :w], p2[:, :w], AF.Exp)
                if nch >= 3:
                    # n-sum: fp16 half-add at 2x, then narrow f32 reduce
                    sv = scr[:, 0:nch * NLOC].rearrange(
                        "p (a h b) -> p a h b", h=2, b=128)
                    nc.vector.tensor_tensor(
                        out=hsum[:, 0:nch * 128].rearrange(
                            "p (a b) -> p a b", b=128),
                        in0=sv[:, :, 0, :], in1=sv[:, :, 1, :], op=OP.add)
                    nc.vector.tensor_reduce(
                        stats[:, ch0:ch0 + nch],
                        hsum[:, 0:nch * 128].rearrange(
                            "p (a b) -> p a b", b=128),
                        axis=AX.X, op=OP.add)
                else:
                    # small groups: single reduce beats add+reduce
                    nc.vector.tensor_reduce(
                        stats[:, ch0:ch0 + nch],
                        scr[:, 0:nch * NLOC].rearrange(
                            "p (a b) -> p a b", b=NLOC),
                        axis=AX.X, op=OP.add)
                return scr

            def psi2_chunk_accum(ch0):
                # single chunk as exp+accumulate on ACT: no DVE work
                p2 = ppool.tile([128, 4 * NLOC], f32, tag="p2")
                nc.tensor.matmul(
                    p2[:, 0:NLOC],
                    lhsT=mega[0:34, ZOFF + ch0 * 128:ZOFF + (ch0 + 1) * 128],
                    rhs=mega[0:34, 0:256],
                    start=True, stop=True)
                scr = spool.tile([128, 4 * NLOC], f16, tag="p2scr")
                nc.scalar.activation(scr[:, 0:NLOC], p2[:, 0:NLOC], AF.Exp,
                                     accum_out=stats[:, ch0:ch0 + 1])

            ch0 = 0
            p1scr = bigpool.tile([128, 2 * M], f16)
            for t, nch in enumerate(GROUPS):
                if nch == 1:
                    psi2_chunk_accum(ch0)
                else:
                    psi2_group(ch0, nch)
                ch0 += nch
                if t == PSI1_GROUP:
                    # psi1 in its own 1-bank PSUM tile + small exp, so
                    # every psi2 tile stays 2-bank and bufs=3 fits
                    p1p = ppools.tile([128, 2 * M], f32, tag="p1p")
                    for c in range(2):
                        nc.tensor.matmul(
                            p1p[:, M * c:M * (c + 1)],
                            lhsT=mega[0:67, 128 * c:128 * (c + 1)],
                            rhs=mega[0:67, 256:320],
                            start=True, stop=True)
                    nc.scalar.activation(p1scr[:, :], p1p[:, :], AF.Exp)
                if t == 3:
                    # stats bulk can go as soon as group 4's reduce lands
                    nc.sync.dma_start(out=st_o[:, 0:14], in_=stats[:, 0:14])
                if t == 3:
                    # A = psi1^T y, two accumulated 256-col matmuls so
                    # the output (and its copy + DMA) stays [64, 256];
                    # the copy rides ACT in the e4->e5 gap (Identity
                    # needs no extra table) and the 32KB fp16 A DMA
                    # completes well before the stats tail.
                    apsum = ppools.tile([M, D], f32, tag="aps")
                    for c in range(2):
                        nc.tensor.matmul(
                            apsum[:, :],
                            lhsT=p1scr[:, M * c:M * (c + 1)],
                            rhs=ybig[:, NLOC * c:NLOC * (c + 1)],
                            start=(c == 0), stop=(c == 1))
                    a_sb = bigpool.tile([M, D], f16)
                    nc.scalar.add(a_sb[:, :], apsum[:, :], 0.0)
                    nc.sync.dma_start(out=a_o[:, :], in_=a_sb[:, :])

            nc.sync.dma_start(out=st_o[:, 14:17], in_=stats[:, 14:17])

    nc.compile()
    return nc


def _get_compiled():
    global _compiled
    if _compiled is None:
        _compiled = _build_bass()
    return _compiled


def _np_softplus(x):
    return np.logaddexp(x, 0.0)


def kernel(y, q_mu, q_log_sigma, z, noise_raw, alpha, variance, _trace=False):
    from concourse.bass_utils import run_bass_kernel_spmd

    nc = _get_compiled()

    f8 = np.float64
    z64 = z.astype(f8)
    al = alpha.astype(f8)
    var = f8(variance[0])
    logvar = np.log(var)

    # z-side stationary block (host-built, replicated to all cores).
    # psi2 is symmetric in (i, j): ship only the 2080 upper-tri pairs.
    iu, ju = np.triu_indices(M)                             # (2080,)
    npairs = iu.shape[0]
    Su = z64[iu] + z64[ju]                                  # (2080, q)
    sqz = (z64[:, None, :] - z64[None, :, :]) ** 2          # (m, m, q)
    s1 = 0.25 * (sqz @ al)                                  # (m, m)
    zl = np.zeros((34, 17 * 128), np.float32)
    zl[0:16, :npairs] = Su.T
    zl[16:32, :npairs] = (-0.25 * Su * Su).T
    zl[32, :npairs] = 1.0
    zl[33, :npairs] = -s1[iu, ju] + 4.0 * logvar

    # per-n exponent features (host elementwise prep) + psi1 z-block
    qmu = q_mu.astype(f8)                                   # (n, q)
    qsig = _np_softplus(q_log_sigma.astype(f8))             # (n, q)
    aq = al[None, :]
    d1 = qsig * aq + 1.0
    d2 = 2.0 * qsig * aq + 1.0
    w1 = aq / d1
    w2 = aq / d2
    h1 = 2.0 * logvar - 0.5 * ((w1 * qmu * qmu).sum(-1)
                               + np.log(d1).sum(-1))        # (n,)

    feat = np.zeros((67, N), np.float32)
    feat[0:16] = (qmu * w2).T
    feat[16:32] = w2.T
    feat[32] = -((qmu * qmu * w2).sum(-1) + 0.5 * np.log(d2).sum(-1))
    feat[33] = 1.0
    feat[34:50] = (qmu * w1).T
    feat[50:66] = w1.T
    feat[66] = h1

    zt = z64.T                                              # (q, m)
    zblock = np.zeros((67, M), np.float32)
    zblock[34:50] = zt
    zblock[50:66] = -0.5 * zt * zt
    zblock[66] = 1.0

    in_maps = []
    for i in range(NCORES):
        r = i * NLOC
        mega = np.zeros((67, 2496), np.float16)
        mega[:, 0:128] = feat[:, r:r + 128]
        mega[:, 128:256] = feat[:, r + 128:r + 256]
        mega[:, 256:320] = zblock
        mega[0:34, 320:2496] = zl
        ybig = np.empty((128, 2 * NLOC), np.float16)
        ybig[:, 0:NLOC] = y[r:r + 128]
        ybig[:, NLOC:2 * NLOC] = y[r + 128:r + 256]
        in_maps.append({"mega": mega, "ybig": ybig})
    br = run_bass_kernel_spmd(nc, in_maps, list(range(NCORES)), trace=_trace)
    res = br.results

    stats = np.zeros((128, 17), f8)
    A = np.zeros((M, D), f8)
    for rr in res:
        stats += rr["out_stats"].astype(f8)
        A += rr["out_A"].astype(f8)

    flat = stats[:, 0:17].T.reshape(17 * 128)
    psi2 = np.empty((M, M), f8)
    psi2[iu, ju] = flat[:npairs]
    psi2[ju, iu] = flat[:npairs]
    tr_yy = float(np.sum(y.astype(f8) ** 2))

    # KL statistics: input-only reductions, done on host like tr_yy
    kl_sum = (-np.log(qsig).sum()
              + 0.5 * ((qsig * qsig).sum() + (qmu * qmu).sum())
              - 0.5 * N * Q)
    kl_term = kl_sum / (N * D)

    # small m x m algebra on host
    k_mm = var * np.exp(-0.5 * (sqz @ al))                  # (m, m)
    noise_var = _np_softplus(f8(noise_raw[0]))
    beta = 1.0 / noise_var
    psi0 = N * var

    cov1 = beta * psi2 + k_mm
    B = np.linalg.solve(cov1, A)
    tr_yWy = beta * tr_yy - np.sum(A * B)

    F = 0.5 * N * np.log(beta)
    F += 0.5 * np.linalg.slogdet(k_mm)[1]
    F -= 0.5 * N * np.log(np.pi)
    F -= 0.5 * np.linalg.slogdet(cov1)[1]
    F -= 0.5 * beta * psi0
    F += 0.5 * np.trace(np.linalg.solve(k_mm, psi2))
    F = (F * D - 0.5 * tr_yWy) / (N * D)

    out = F - kl_term
    result = np.asarray(out, dtype=np.float32)
    if _trace:
        return result, br
    return result
